# revision 1
# baseline (speedup 1.0000x reference)
"""CycleFC (per-channel width-shift + 1x1 conv) Trainium2 kernel.

Full shapes: x [32, 256, 56, 56] f32, weight [256, 256], bias [256].
out[b,o,h,w] = sum_c weight[o,c] * shift(x)[b,c,h,w] + bias[o]
where shift moves channel c along width by off(c) = (c+3)%7-3, zero-padded.

Strategy
--------
- Data-parallel over batch: 8 cores x 4 batches.
- Channels are permuted host-side so equal-shift channels ("classes",
  c mod 7) are contiguous; the weight matrix is permuted to match, so the
  contraction is order-invariant and no output un-permutation is needed.
- Per (batch, channel-group-of-128) SBUF tile [128, 3136]: one DMA per
  shift class loads the shifted window straight from DRAM (the shift is
  just a width-offset in the source AP); the zeroed edge columns are
  memset once up front. This keeps every matmul at full K=128.
- Matmul: out[o, hw] = lhsT.T @ rhs with lhsT = permuted weight.T
  [C, O] and rhs = shifted x tile [C, hw-chunk]. N-chunks of 448 (8 rows),
  accumulated over the 2 channel groups in PSUM. Bias is fused into the
  PSUM->SBUF eviction on the scalar engine.

Semaphore-wait budget
---------------------
The walrus codegen encodes at most ONE semaphore wait per instruction
(any opcode), and a DMA on a reused DMAHW/DMASW lane always spends it on
an own-lane wait. The whole kernel is therefore structured so every
instruction has at most one un-observed dependency when it issues:
- x tiles are unique (never recycled) and their zero edge-bands are
  memset once up front, so class-load DMAs carry no data waits;
- 8 dummy SBUF->SBUF DMAs "prime" all SWDGE lanes with a wait on the
  last memset, so later loads never need a DVE wait;
- tiny "absorber" matmuls advance PE's observed clock one writer at a
  time before the real matmuls touch a tile;
- evictions all run on the scalar engine so the 8 output stores (on 8
  fresh HWDGE lanes) wait on ACT alone.
"""

import numpy as np

B, C, O, H, W = 32, 256, 256, 56, 56
KS = 7
PAD = KS // 2
N_CORES = 8
B_LOC = B // N_CORES
HW = H * W
ROWS_PER_CHUNK = 8
N_FREE = ROWS_PER_CHUNK * W  # 448
N_CHUNKS = H // ROWS_PER_CHUNK  # 7
USE_F32R = False

_OFFS = [(r + PAD) % KS - PAD for r in range(KS)]  # [0,1,2,3,-3,-2,-1]


def _perm_and_segments(n_chan):
    """Channel permutation (sort by c mod 7) and per-128-group DMA segments.

    Returns (perm, segs) where segs[g] is a list of
    (off, p0, p1, ch_start, ch_stop) : local partitions [p0,p1) of group g
    hold original channels ch_start:ch_stop:KS, shifted by `off`.
    """
    mods = np.arange(n_chan) % KS
    perm = np.argsort(mods, kind="stable")
    counts = [int(np.sum(mods == r)) for r in range(KS)]
    starts = np.concatenate([[0], np.cumsum(counts)])
    n_groups = n_chan // 128
    segs = [[] for _ in range(n_groups)]
    for r in range(KS):
        cs, ce = int(starts[r]), int(starts[r + 1])
        for g in range(n_groups):
            s, e = max(cs, g * 128), min(ce, (g + 1) * 128)
            if s >= e:
                continue
            j0, j1 = s - cs, e - cs
            segs[g].append((_OFFS[r], s - g * 128, e - g * 128,
                            r + KS * j0, r + KS * (j1 - 1) + 1))
    return perm, segs


def build_nc(b_loc=B_LOC, n_chan=C, n_out=O, h=H, w=W, rows_per_chunk=ROWS_PER_CHUNK,
             use_f32r=USE_F32R, psum_bufs=6):
    import concourse.bass as bass
    import concourse.mybir as mybir
    from concourse.tile import TileContext

    f32 = mybir.dt.float32
    # fp32r runs the PE at 4x the fp32 rate (1 cycle/row for N>=256). The
    # BIR verifier requires fp32r matmul inputs to be *produced* rounded,
    # so the x/weight tiles are float32r-dtyped and the SWDGE load DMAs
    # cast f32 -> f32r in flight.
    xdt = mybir.dt.float32r if use_f32r else f32
    hw = h * w
    n_free = rows_per_chunk * w
    n_chunks = h // rows_per_chunk
    assert h % rows_per_chunk == 0
    n_groups = n_chan // 128
    o_groups = n_out // 128
    _, segs = _perm_and_segments(n_chan)

    nc = bass.Bass()
    x_d = nc.declare_dram_parameter("x", [b_loc, n_chan, h, w], f32, isOutput=False)
    w_d = nc.declare_dram_parameter("wt", [n_chan, n_out], f32, isOutput=False)
    b_d = nc.declare_dram_parameter("bias", [128, o_groups], f32, isOutput=False)
    out_d = nc.declare_dram_parameter("out", [b_loc, n_out, h, w], f32, isOutput=True)

    from concourse.tile import add_dep_helper

    funnel = []  # final instruction of every proc, for the drain funnel

    with TileContext(nc) as tc:
        with (
            tc.tile_pool(name="const", bufs=1) as cpool,
            tc.tile_pool(name="xp", bufs=1) as xpool,
            tc.tile_pool(name="op", bufs=1) as opool,
            tc.tile_pool(name="ps", bufs=psum_bufs, space="PSUM") as pspool,
            tc.tile_pool(name="jk", bufs=1, space="PSUM") as jkpool,
        ):
            # --- unique x tiles + one-time edge-band memsets -------------
            xts = []
            last_band = None
            for b in range(b_loc):
                row = []
                for g in range(n_groups):
                    xt = xpool.tile([128, hw], xdt, tag=f"x{b}_{g}")
                    xt3 = xt[:].rearrange("p (h w) -> p h w", w=w)
                    nc.vector.memset(xt3[:, :, 0:PAD], 0.0)
                    last_ms = nc.vector.memset(xt3[:, :, w - PAD:w], 0.0)
                    last_band = xt
                    row.append(xt)
                xts.append(row)

            def data_win(off):
                # columns holding DMA data for a class with shift `off`
                return (max(0, -off), w - max(0, off))

            # A probe element of the last-memset tile that no class DMA
            # ever overwrites (so reads of it only ever depend on the
            # memsets). Partition start must be 0/32/64/96 aligned.
            probe_pc = None
            for p_align in (0, 32, 64, 96):
                for (off, p0, p1, _, _) in segs[n_groups - 1]:
                    if p0 <= p_align < p1:
                        lo, hi = data_win(off)
                        for col in (0, w - 1):
                            if not (lo <= col < hi):
                                probe_pc = (p_align, col)
                        break
                if probe_pc:
                    break
            assert probe_pc, "no pure-band probe element"
            pp, pcol = probe_pc
            band_el = last_band[pp:pp + 1, pcol:pcol + 1]

            # --- prime all 8 SWDGE lanes on the final memset -------------
            sjunk = cpool.tile([128, 16], xdt, tag="sjunk")
            for i in range(8):
                nc.gpsimd.dma_start(out=sjunk[0:1, i:i + 1], in_=band_el)

            # --- constants (SWDGE; lanes reused -> own-lane wait only) ---
            wtiles = []
            for g in range(n_groups):
                wt = cpool.tile([128, n_out], xdt, tag=f"w{g}")
                nc.gpsimd.dma_start(out=wt[:], in_=w_d[g * 128:(g + 1) * 128, :])
                wtiles.append(wt)
            btile = cpool.tile([128, o_groups], f32, tag="bias")
            nc.gpsimd.dma_start(out=btile[:], in_=b_d[:])

            # --- PE absorbers --------------------------------------------
            jk = jkpool.tile([32, 512], f32, tag="junk")
            jk_col = [0]

            def absorb(lhsT, rhs, pos):
                nfree = rhs.shape[-1]
                c = jk_col[0]
                jk_col[0] = c + 2
                assert jk_col[0] <= 512
                m = min(lhsT.shape[-1], 32)
                nc.tensor.matmul(jk[0:m, c:c + nfree], lhsT, rhs, start=True,
                                 stop=True, skip_group_check=True,
                                 tile_position=(pos, 0))

            # memsets (DVE) -> PE: probe reads a band element of the last
            # memset; DVE sem is cumulative so this covers all memsets.
            absorb(band_el, band_el, pp)
            # const DMA lanes -> PE
            absorb(wtiles[0][0:32, 0:32], wtiles[0][0:32, 32:34], 0)
            absorb(wtiles[0][0:32, 0:32], wtiles[1][0:32, 0:2], 0)

            # bias lane -> ACT: probe on the scalar engine so evictions
            # never wait on the bias DMA.
            ajunk = cpool.tile([128, 4], f32, tag="ajunk")
            nc.scalar.activation(ajunk[0:32, 0:1], btile[0:32, 0:1],
                                 mybir.ActivationFunctionType.Identity)

            # The DMA lowering fails at runtime when a strided transfer has
            # more than 27 row-chunks per partition; shifted-class loads
            # ([nch, h, w-window]) are split into row pieces of <=27.
            n_pieces = -(-h // 27)
            base = h // n_pieces
            rem = h % n_pieces
            h_pieces = []
            r = 0
            for i in range(n_pieces):
                r2 = r + base + (1 if i < rem else 0)
                h_pieces.append((r, r2))
                r = r2

            def absorb_tile(xt, g):
                # per class-DMA piece: probe col in this class's data window
                # and not in any later (unobserved) overlapping class's
                # window; flat index row-offset selects the piece.
                spans = [(p, p + 32) for p in range(0, 128, 32)]
                for i, (off, p0, p1, _, _) in enumerate(segs[g]):
                    lo, hi = data_win(off)
                    pick = None
                    for (s0, s1) in spans:
                        if not (p0 < s1 and p1 > s0):
                            continue
                        for col in range(lo, hi):
                            ok = True
                            for j, (off2, q0, q1, _, _) in enumerate(segs[g]):
                                if j <= i or not (q0 < s1 and q1 > s0):
                                    continue
                                lo2, hi2 = data_win(off2)
                                if lo2 <= col < hi2:
                                    ok = False
                                    break
                            if ok:
                                pick = (s0, s1, col)
                                break
                        if pick:
                            break
                    assert pick, f"no probe col for seg {i} of group {g}"
                    s0, s1, col = pick
                    pieces = [(0, h)] if off == 0 else h_pieces
                    for (r0, _) in pieces:
                        fi = r0 * w + col
                        absorb(wtiles[0][s0:s1, 0:32], xt[s0:s1, fi:fi + 1],
                               s0)

            # --- main loop ----------------------------------------------
            sw_dmas = []
            last_mm = last_act = None
            for b in range(b_loc):
                for g in range(n_groups):
                    xt3 = xts[b][g][:].rearrange("p (h w) -> p h w", w=w)
                    for (off, p0, p1, c0, c1) in segs[g]:
                        if off == 0:
                            d = nc.gpsimd.dma_start(
                                out=xt3[p0:p1, :, :],
                                in_=x_d[b, c0:c1:KS, :, :])
                            sw_dmas.append(d)
                            continue
                        for (r0, r1) in h_pieces:
                            if off > 0:
                                d = nc.gpsimd.dma_start(
                                    out=xt3[p0:p1, r0:r1, 0:w - off],
                                    in_=x_d[b, c0:c1:KS, r0:r1, off:w])
                            else:
                                d = nc.gpsimd.dma_start(
                                    out=xt3[p0:p1, r0:r1, -off:w],
                                    in_=x_d[b, c0:c1:KS, r0:r1, 0:w + off])
                            sw_dmas.append(d)
                    absorb_tile(xts[b][g], g)

                for og in range(o_groups):
                    ot = opool.tile([128, hw], f32, tag=f"ot{b}_{og}")
                    for n in range(n_chunks):
                        nsl = slice(n * n_free, (n + 1) * n_free)
                        ps = pspool.tile([128, n_free], f32, tag="ps")
                        for g in range(n_groups):
                            lhsT = wtiles[g][:, og * 128:(og + 1) * 128]
                            rhs = xts[b][g][:, nsl]
                            last_mm = nc.tensor.matmul(
                                ps[:], lhsT, rhs, start=(g == 0),
                                stop=(g == n_groups - 1))
                        last_act = nc.scalar.activation(
                            ot[:, nsl], ps[:],
                            mybir.ActivationFunctionType.Identity,
                            bias=btile[:, og:og + 1])
                    st = nc.sync.dma_start(
                        out=out_d[b, og * 128:(og + 1) * 128].rearrange(
                            "c h w -> c (h w)"),
                        in_=ot[:])
                    funnel.append(st)

            # The end-of-kernel drain would otherwise carry a wait for every
            # live proc (~19 > the 1-wait encoding limit). Funnel: SP nops
            # each waiting on one outstanding producer, so the drain's
            # requirements are already observed on SP.
            funnel.extend(sw_dmas[-8:])
            funnel.append(last_mm)
            funnel.append(last_act)
            funnel.append(last_ms)
            for dep in funnel:
                nop = nc.sync.nop(nofuse=True, hint="drain_funnel")
                add_dep_helper(nop.ins, dep.ins, reason="drain funnel")
    return nc


_CACHED_NC = None


def _get_nc():
    global _CACHED_NC
    if _CACHED_NC is None:
        _CACHED_NC = build_nc()
    return _CACHED_NC


def run(x, weight, bias, trace=False):
    from concourse.bass_utils import run_bass_kernel_spmd

    perm, _ = _perm_and_segments(C)
    wt = np.ascontiguousarray(weight[:, perm].T)          # [C_perm, O]
    b2 = np.ascontiguousarray(bias.reshape(O // 128, 128).T)  # [128, o_groups]
    x = np.ascontiguousarray(x, dtype=np.float32)

    nc = _get_nc()
    in_maps = [
        {"x": x[i * B_LOC:(i + 1) * B_LOC], "wt": wt, "bias": b2}
        for i in range(N_CORES)
    ]
    res = run_bass_kernel_spmd(nc, in_maps, list(range(N_CORES)), trace=trace)
    out = np.concatenate([res.results[i]["out"] for i in range(N_CORES)], axis=0)
    return out, res


def kernel(x, weight, bias):
    out, _ = run(x, weight, bias, trace=False)
    return out

